# revision 11
# baseline (speedup 1.0000x reference)
"""v7 (7 matmul passes/col/step): fp16 everywhere on the matmul path; single packed weight blob;
single fp16 x tensor (one DMA per t,slab); only output = bf16(mem2-1)
via SWDGE cast-DMA; spk2 reconstructed on host from the sign.

Math per step t (threshold 1, decay beta), state m2' = mem2 - 1:
  ps1 = x_t @ W1h + x_t @ W1l + ones*(-1/2) + s1_{t-1} @ (-I/2)   [PE fp16]
  m1  = beta*m1 + ps1                                             [DVE fp32]
  s1  = Sign(m1 - 1) in {-1,+1}
  ps2 = s1 @ (W2h+W2l) + ones*(sum w2/2 - 1/2 + beta-1) + s2 @ (-I/2)
  m2' = beta*m2' + ps2          -> out bf16 (cast in DMA)
  s2  = Sign(m2')
Host: spk2 = (m2c > 0), mem2 = m2c + 1.   (bf16 cast preserves sign)

Precision: fp16 hi/lo weights are near-exact (2^-21); x fp16 adds a
~2^-11 random walk; emulated total rel err ~1.0e-2 < 2e-2 gate.

Lane layout per core: 2 slabs x 21 lanes x NCOLS columns (as v4).
"""

import numpy as np
from contextlib import ExitStack
from concurrent.futures import ThreadPoolExecutor

T = 10
NI, NH, NO = 4, 5, 3
BETA = 0.95
B_FULL = 1_000_000
NCORES = 8

NBL = 21
NSLAB = 2
NCOLS = 2978
NPB = 1024
BC = NSLAB * NBL * NCOLS  # 125,076
BPAD = BC * NCORES        # 1,000,608

XR = NBL * NI             # 84
M1 = NBL * NH             # 105
M2 = NBL * NO             # 63
M2P = 2 * M2              # 126

# weight blob column offsets
C_W1H = 0
C_W1L = 128
C_R1 = 256
C_W2HA = 384
C_W2LA = 510
C_W2HB = 636
C_W2LB = 762
C_R2 = 888
WCOLS = 1014

bass_mult = None
bass_add = None

# timing experiments: run the whole T-loop this many times (outputs are
# overwritten by later passes; correctness only valid for REPEAT=1)
REPEAT = 1


def _init_ops():
    global bass_mult, bass_add
    import concourse.mybir as mybir
    bass_mult = mybir.AluOpType.mult
    bass_add = mybir.AluOpType.add


def fp16_split(a):
    hi = a.astype(np.float16)
    lo = (a.astype(np.float32) - hi.astype(np.float32)).astype(np.float16)
    return hi, lo


def make_weight_blob(w1, w2):
    w1 = np.asarray(w1, np.float64)
    w2 = np.asarray(w2, np.float64)
    wb = np.zeros((128, WCOLS), np.float32)
    # w1 block-diagonal: rows (bl,i) -> cols (5bl+h)
    w1f = np.zeros((XR, 128), np.float32)
    for bl in range(NBL):
        for i in range(NI):
            for h in range(NH):
                w1f[4 * bl + i, 5 * bl + h] = w1[h, i]
    w1h, w1l = fp16_split(w1f)
    wb[0:XR, C_W1H : C_W1H + 128] = w1h.astype(np.float32)
    wb[0:XR, C_W1L : C_W1L + 128] = w1l.astype(np.float32)
    # r1 = -I/2 over M1, plus the -1/2 threshold const on s1's ones row
    wb[0:M1, C_R1 : C_R1 + M1] = -0.5 * np.eye(M1, dtype=np.float32)
    wb[M1, C_R1 : C_R1 + M1] = -0.5
    # w2 pair-packed per slab, ones-row const includes m2' recentering
    c2 = w2.sum(axis=1) / 2.0 - 0.5 + (BETA - 1.0)
    for s, (ch, cl) in enumerate(((C_W2HA, C_W2LA), (C_W2HB, C_W2LB))):
        w2f = np.zeros((M1 + 1, M2P), np.float32)
        for bl in range(NBL):
            for h in range(NH):
                for o in range(NO):
                    w2f[5 * bl + h, 63 * s + 3 * bl + o] = w2[o, h] / 2.0
        for bl in range(NBL):
            for o in range(NO):
                w2f[M1, 63 * s + 3 * bl + o] = c2[o]
        w2h, w2l = fp16_split(w2f)
        wb[0 : M1 + 1, ch : ch + M2P] = w2h.astype(np.float32)
        wb[0 : M1 + 1, cl : cl + M2P] = w2l.astype(np.float32)
    # r2 = -I/2 over M2P
    wb[0:M2P, C_R2 : C_R2 + M2P] = -0.5 * np.eye(M2P, dtype=np.float32)
    return wb.astype(np.float16)


def _split_multi_waits(nc):
    """Walrus accepts only ONE sync-wait per compute instruction; hoist
    extras onto pure-sync EventSemaphore instructions."""
    import concourse.mybir as mybir

    for f in nc.m.functions:
        for blk in f.blocks:
            out = []
            for ins in blk.instructions:
                si = ins.sync_info
                if (
                    si is not None
                    and len(si.on_wait) > 1
                    and not isinstance(ins, mybir.InstEventSemaphore)
                ):
                    waits = list(si.on_wait)
                    for j, w in enumerate(waits[:-1]):
                        out.append(
                            mybir.InstEventSemaphore(
                                name=f"{ins.name}-ws{j}",
                                engine=ins.engine,
                                ins=[],
                                outs=[],
                                sync_info=mybir.SyncInfo(
                                    on_wait=[w], on_update=[]
                                ),
                            )
                        )
                    ins.sync_info = mybir.SyncInfo(
                        on_wait=[waits[-1]], on_update=list(si.on_update)
                    )
                out.append(ins)
            blk.instructions = out


def build_nc(split_waits=True):
    _init_ops()
    import concourse.bass as bass
    import concourse.mybir as mybir
    from concourse.tile import TileContext

    f32 = mybir.dt.float32
    f16 = mybir.dt.float16
    bf16 = mybir.dt.bfloat16
    Act = mybir.ActivationFunctionType

    groups = []
    c0 = 0
    while c0 < NCOLS:
        n = min(NPB, NCOLS - c0)
        groups.append((c0, n))
        c0 += n

    nc = bass.Bass()
    x_d = nc.declare_dram_parameter("x", [T, NSLAB, XR, NCOLS], f16,
                                    isOutput=False)
    wb_d = nc.declare_dram_parameter("wb", [128, WCOLS], f16, isOutput=False)
    mem_d = nc.declare_dram_parameter("mem2c", [T, M2P, NCOLS], bf16,
                                      isOutput=True)

    with ExitStack() as ctx:
        tc = ctx.enter_context(TileContext(nc))
        wp = ctx.enter_context(tc.tile_pool(name="wp", bufs=1))
        st = ctx.enter_context(tc.tile_pool(name="st", bufs=1))
        xp = ctx.enter_context(tc.tile_pool(name="xp", bufs=1))
        ps = ctx.enter_context(tc.tile_pool(name="ps", bufs=2, space="PSUM"))

        wb = wp.tile([128, WCOLS], f16, tag="wb")
        nc.sync.dma_start(wb[:], wb_d[:])
        negone = wp.tile([128, 1], f32, tag="negone")
        nc.vector.memset(negone[:], -1.0)
        zerob = wp.tile([128, 1], f32, tag="zerob")
        nc.vector.memset(zerob[:], 0.0)

        # x ring: 3 buffers per slab; row XR = ones (set once)
        NRING = 3
        xs = [[xp.tile([XR, NCOLS], f16, tag=f"x_{s}_{r}",
                       name=f"x_{s}_{r}") for r in range(NRING)]
              for s in range(NSLAB)]

        # per-chunk state tiles
        s1t = [[st.tile([M1 + 1, n], f16, tag=f"s1_{s}_{gi}",
                        name=f"s1_{s}_{gi}") for gi, (c0, n) in enumerate(groups)]
               for s in range(NSLAB)]
        m1t = [[st.tile([M1, n], f32, tag=f"m1_{s}_{gi}",
                        name=f"m1_{s}_{gi}") for gi, (c0, n) in enumerate(groups)]
               for s in range(NSLAB)]
        s2t = [st.tile([M2P, n], f16, tag=f"s2_{gi}", name=f"s2_{gi}")
               for gi, (c0, n) in enumerate(groups)]
        m2t = [st.tile([M2P, n], f32, tag=f"m2_{gi}", name=f"m2_{gi}")
               for gi, (c0, n) in enumerate(groups)]
        # Only s1's ones row (105) must be live before the first L2 matmul;
        # rows 96:105 are overwritten by the first ACT before that read
        # (DVE needs a 32-aligned partition base, hence the wider memset).
        # Everything else (s1/-1, m1/0, s2/-1, m2/-1) is made redundant by
        # the t=0 special case below: reset passes are skipped at t=0 and
        # the t=0 DVE ops don't read prior state.
        for s in range(NSLAB):
            for gi, (c0, n) in enumerate(groups):
                nc.vector.memset(s1t[s][gi][96 : M1 + 1, :], 1.0)

        def mm(out_ap, w_ap, rhs_ap, start, stop):
            n = out_ap.shape[-1]
            o = 0
            while o < n:
                k = min(512, n - o)
                nc.tensor.matmul(
                    out_ap[:, o : o + k], w_ap, rhs_ap[:, o : o + k],
                    start=start, stop=stop,
                )
                o += k

        for rep in range(REPEAT):
          for t in range(T):
            ring = (rep * T + t) % NRING
            for s in range(NSLAB):
                nc.sync.dma_start(xs[s][ring][0:XR, :], x_d[t, s, :, :])
            for gi, (c0, n) in enumerate(groups):
                cs = slice(c0, c0 + n)
                first = (rep == 0 and t == 0)
                for s in range(NSLAB):
                    xv = xs[s][ring]
                    ps1 = ps.tile([128, n], f32, tag="ps1",
                                  name=f"ps1_{rep}_{t}_{gi}_{s}")
                    mm(ps1[:, 0:n], wb[0:XR, C_W1H : C_W1H + 128],
                       xv[:, cs], start=True, stop=first)
                    if not first:
                        # reset + threshold const ride on s1 (incl. its
                        # ones row); at t=0 reset is exactly zero
                        mm(ps1[:, 0:n], wb[0 : M1 + 1, C_R1 : C_R1 + 128],
                           s1t[s][gi][:, 0:n], start=False, stop=True)
                        nc.vector.scalar_tensor_tensor(
                            m1t[s][gi][:, 0:n], m1t[s][gi][:, 0:n], BETA,
                            ps1[0:M1, 0:n], bass_mult, bass_add,
                        )
                    else:
                        nc.vector.tensor_copy(
                            m1t[s][gi][:, 0:n], ps1[0:M1, 0:n])
                    nc.scalar.activation(
                        s1t[s][gi][0:M1, 0:n], m1t[s][gi][:, 0:n],
                        Act.Sign, bias=negone[0:M1, :],
                    )
                ps2 = ps.tile([M2P, n], f32, tag="ps2", name=f"ps2_{rep}_{t}_{gi}")
                mm(ps2[:, 0:n], wb[0 : M1 + 1, C_W2HA : C_W2HA + M2P],
                   s1t[0][gi][:, 0:n], start=True, stop=False)
                mm(ps2[:, 0:n], wb[0 : M1 + 1, C_W2HB : C_W2HB + M2P],
                   s1t[1][gi][:, 0:n], start=False, stop=first)
                if not first:
                    mm(ps2[:, 0:n], wb[0:M2P, C_R2 : C_R2 + M2P],
                       s2t[gi][:, 0:n], start=False, stop=True)
                    nc.vector.scalar_tensor_tensor(
                        m2t[gi][:, 0:n], m2t[gi][:, 0:n], BETA, ps2[:, 0:n],
                        bass_mult, bass_add,
                    )
                else:
                    # mem2_0 = cur2_0; m2' = ps2 + (1/2 - beta)
                    nc.vector.tensor_scalar(
                        m2t[gi][:, 0:n], ps2[:, 0:n], 0.5 - BETA, None,
                        bass_add,
                    )
                if not (rep == REPEAT - 1 and t == T - 1):
                    # s2 is dead after the last step
                    nc.scalar.activation(
                        s2t[gi][:, 0:n], m2t[gi][:, 0:n], Act.Sign,
                        bias=zerob[0:M2P, :],
                    )
                # bf16 output cast happens inside the SWDGE DMA
                nc.gpsimd.dma_start(mem_d[t, :, cs], m2t[gi][:, 0:n])

    if split_waits:
        _split_multi_waits(nc)
    return nc


def prep_core_x(xpad, c):
    xc = xpad[:, c * BC : (c + 1) * BC, :].reshape(T, NSLAB, NBL, NCOLS, NI)
    xc = np.ascontiguousarray(xc.transpose(0, 1, 2, 4, 3)).reshape(
        T, NSLAB, XR, NCOLS
    )
    return xc.astype(np.float16)


def unpack_outputs(res_c):
    m2c = res_c["mem2c"]   # [T, M2P, NCOLS] bf16
    out_s = np.empty((T, BC, NO), np.float32)
    out_m = np.empty((T, BC, NO), np.float32)
    v_s = out_s.reshape(T, NSLAB, NBL, NCOLS, NO)
    v_m = out_m.reshape(T, NSLAB, NBL, NCOLS, NO)
    for s in range(NSLAB):
        rows = slice(63 * s, 63 * s + M2)
        b = m2c[:, rows, :].astype(np.float32).reshape(
            T, NBL, NO, NCOLS).transpose(0, 1, 3, 2)
        v_s[:, s] = (b > 0.0).astype(np.float32)
        v_m[:, s] = b + 1.0
    return out_s, out_m


def kernel(**inputs):
    x = np.asarray(inputs["x"], dtype=np.float32)
    w1 = np.asarray(inputs["w1"], dtype=np.float32)
    w2 = np.asarray(inputs["w2"], dtype=np.float32)

    from concourse.bass_utils import run_bass_kernel_spmd

    nc = build_nc()
    wb = make_weight_blob(w1, w2)

    xpad = np.zeros((T, BPAD, NI), dtype=np.float32)
    xpad[:, :B_FULL] = x
    with ThreadPoolExecutor(8) as ex:
        xs = list(ex.map(lambda c: prep_core_x(xpad, c), range(NCORES)))
    in_maps = [{"x": xs[c], "wb": wb} for c in range(NCORES)]

    import time as _time
    _t0 = _time.time()
    res = run_bass_kernel_spmd(nc, in_maps, list(range(NCORES))).results
    print(f"[kernel7] device compile+run {_time.time()-_t0:.1f}s", flush=True)

    spk2 = np.empty((T, BPAD, NO), dtype=np.float32)
    mem2 = np.empty((T, BPAD, NO), dtype=np.float32)

    def fill(c):
        s, m = unpack_outputs(res[c])
        spk2[:, c * BC : (c + 1) * BC] = s
        mem2[:, c * BC : (c + 1) * BC] = m

    with ThreadPoolExecutor(8) as ex:
        list(ex.map(fill, range(NCORES)))
    return spk2[:, :B_FULL], mem2[:, :B_FULL]
